# revision 1
# baseline (speedup 1.0000x reference)
"""Trainium2 Bass kernel for nn_DecorrelatedReNorm_17231408791729.

Math: the reference computes
    out = (X_c @ W @ W_inv + X_mean - running_mean) @ running_W
with W = U diag(S^-1/2) U^T and W_inv = U diag(S^1/2) U^T from eigh(cov).
W @ W_inv == I exactly (same eigenbasis), and X_c + X_mean == X, so
    out = (X - running_mean) @ running_W
identically; the eigh chain contributes only fp32 rounding (~1e-6 rel).

Strategy (data-parallel over N across 8 cores):
  - host: shard X rows 8 ways; transpose each shard to [C, rows] so the
    contraction dim (C) lands on SBUF partitions with contiguous DMAs;
    fold running_mean into a bias vector  b = -(running_mean @ running_W).
  - device (per core): for each 512-row macro-tile, stream X^T slab in,
    16 fp32 matmuls (K=4x128 chunks, N=512) accumulate in PSUM, DVE adds
    the broadcast bias while copying PSUM->SBUF, stream out.
  - host: concatenate the 8 row shards.
"""

import numpy as np
from contextlib import ExitStack

import concourse.bass as bass
import concourse.tile as tile
from concourse import bacc, mybir
from concourse.bass_utils import run_bass_kernel_spmd
from concourse.masks import make_identity

C = 512
N_ROWS = 131072
N_CORES = 8
ROWS_PER_CORE = N_ROWS // N_CORES  # 16384
R_TILE = 512                       # rows per macro-tile
P = 128
KC = C // P                        # 4 contraction chunks
JT = R_TILE // P                   # 4 row sub-chunks per macro-tile


def build_bass(nrows: int = ROWS_PER_CORE, mm_dt=None, reps: int = 1):
    mm_dt = mm_dt if mm_dt is not None else mybir.dt.float32
    nc = bacc.Bacc(
        "TRN2",
        target_bir_lowering=False,
        debug=False,
        enable_asserts=False,
    )
    xt = nc.dram_tensor("xt", [C, nrows], mm_dt, kind="ExternalInput").ap()
    w = nc.dram_tensor("w", [C, C], mm_dt, kind="ExternalInput").ap()
    b = nc.dram_tensor("bias", [1, C], mybir.dt.float32, kind="ExternalInput").ap()
    out = nc.dram_tensor(
        "out", [nrows, C], mybir.dt.float32, kind="ExternalOutput"
    ).ap()

    t_count = nrows // R_TILE
    # [T, p, kc, r]: partition = cin within chunk, free = (chunk, row)
    xt_r = xt.rearrange("(kc p) (t r) -> t p kc r", p=P, r=R_TILE)
    # [p, kc, n]: partition = cin within chunk, free = (chunk, cout)
    w_r = w.rearrange("(kc p) n -> p kc n", p=P)
    # [T, p, j, n]: partition = row within sub-chunk, free = (sub-chunk, cout)
    out_r = out.rearrange("(t j p) n -> t p j n", j=JT, p=P)

    with tile.TileContext(nc) as tc, ExitStack() as ctx:
        singles = ctx.enter_context(tc.tile_pool(name="singles", bufs=1))
        xpool = ctx.enter_context(tc.tile_pool(name="x", bufs=3))
        opool = ctx.enter_context(tc.tile_pool(name="o", bufs=3))
        pspool = ctx.enter_context(tc.tile_pool(name="ps", bufs=8, space="PSUM"))

        w_tile = singles.tile([P, KC, C], mm_dt)
        nc.sync.dma_start(out=w_tile[:], in_=w_r)
        bias_tile = singles.tile([P, C], mybir.dt.float32)
        b_bcast = bass.AP(tensor=b.tensor, offset=b.offset, ap=[[0, P], [1, C]])
        nc.sync.dma_start(out=bias_tile[:], in_=b_bcast)

        for _ in range(reps):
            for t in range(t_count):
                x_tile = xpool.tile([P, KC, R_TILE], mm_dt, tag="x")
                nc.sync.dma_start(out=x_tile[:], in_=xt_r[t])
                o_tile = opool.tile([P, JT, C], mybir.dt.float32, tag="o")
                for j in range(JT):
                    ps = pspool.tile([P, C], mybir.dt.float32, tag="ps")
                    for k in range(KC):
                        nc.tensor.matmul(
                            ps[:],
                            x_tile[:, k, bass.ts(j, P)],
                            w_tile[:, k, :],
                            start=(k == 0),
                            stop=(k == KC - 1),
                        )
                    nc.vector.tensor_add(o_tile[:, j, :], ps[:], bias_tile[:])
                nc.sync.dma_start(out=out_r[t], in_=o_tile[:])

    nc.compile()
    return nc


def build_bass_exact(
    nrows: int = ROWS_PER_CORE,
    reps: int = 1,
    out_dma: str = "sync",
    bufs_x: int = 3,
    bufs_o: int = 3,
    r_tile_rows: int = R_TILE,
):
    """out = X + X @ (W - I) + bias, with the residual matmul in float32r.

    X rides the exact fp32 path (DVE add); the float32r truncation only
    touches the residual term, which is exactly zero when W == I. Input X
    is the natural [rows, C] layout; X^T tiles for the matmul are made
    on-chip with PE transposes.
    """
    f32, f32r = mybir.dt.float32, mybir.dt.float32r
    nc = bacc.Bacc(
        "TRN2",
        target_bir_lowering=False,
        debug=False,
        enable_asserts=False,
    )
    x = nc.dram_tensor("x", [nrows, C], f32, kind="ExternalInput").ap()
    r = nc.dram_tensor("r", [C, C], f32r, kind="ExternalInput").ap()
    b = nc.dram_tensor("bias", [1, C], f32, kind="ExternalInput").ap()
    out = nc.dram_tensor("out", [nrows, C], f32, kind="ExternalOutput").ap()

    JT = r_tile_rows // P
    out_eng = {"sync": nc.sync, "scalar": nc.scalar, "gpsimd": nc.gpsimd}[out_dma]
    t_count = nrows // r_tile_rows
    # [T, p, j, c]: partition = row within sub-chunk, free = (sub-chunk, col)
    x_r = x.rearrange("(t j p) c -> t p j c", j=JT, p=P)
    r_r = r.rearrange("(kc p) n -> p kc n", p=P)
    out_r = out.rearrange("(t j p) n -> t p j n", j=JT, p=P)

    with tile.TileContext(nc) as tc, ExitStack() as ctx:
        singles = ctx.enter_context(tc.tile_pool(name="singles", bufs=1))
        xpool = ctx.enter_context(tc.tile_pool(name="x", bufs=bufs_x))
        xtpool = ctx.enter_context(tc.tile_pool(name="xt", bufs=4))
        opool = ctx.enter_context(tc.tile_pool(name="o", bufs=bufs_o))
        pst_pool = ctx.enter_context(tc.tile_pool(name="pst", bufs=4, space="PSUM"))
        pso_pool = ctx.enter_context(tc.tile_pool(name="pso", bufs=4, space="PSUM"))

        r_tile = singles.tile([P, KC, C], f32r)
        nc.sync.dma_start(out=r_tile[:], in_=r_r)
        bias_tile = singles.tile([P, C], f32)
        b_bcast = bass.AP(tensor=b.tensor, offset=b.offset, ap=[[0, P], [1, C]])
        nc.sync.dma_start(out=bias_tile[:], in_=b_bcast)
        ident = singles.tile([P, P], f32)
        make_identity(nc, ident[:])

        for _ in range(reps):
            for t in range(t_count):
                x_tile = xpool.tile([P, JT, C], f32, tag="x")
                nc.sync.dma_start(out=x_tile[:], in_=x_r[t])
                o_tile = opool.tile([P, JT, C], f32, tag="o")
                for j in range(JT):
                    ps_t = pst_pool.tile([P, KC, P], f32, tag="pst")
                    for k in range(KC):
                        nc.tensor.transpose(
                            ps_t[:, k, :],
                            x_tile[:, j, bass.ts(k, P)],
                            ident[:],
                        )
                    # fp32 -> float32r rounding happens in this DVE copy
                    xT = xtpool.tile([P, KC, P], f32r, tag="xt")
                    nc.vector.tensor_copy(xT[:], ps_t[:])
                    ps_o = pso_pool.tile([P, C], f32, tag="pso")
                    for k in range(KC):
                        nc.tensor.matmul(
                            ps_o[:],
                            xT[:, k, :],
                            r_tile[:, k, :],
                            start=(k == 0),
                            stop=(k == KC - 1),
                        )
                    nc.vector.tensor_add(o_tile[:, j, :], ps_o[:], x_tile[:, j, :])
                    nc.gpsimd.tensor_add(o_tile[:, j, :], o_tile[:, j, :], bias_tile[:])
                out_eng.dma_start(out=out_r[t], in_=o_tile[:])

    nc.compile()
    return nc


_CACHE: dict = {}


def _prep_in_maps(X, running_mean, running_W):
    """Inputs for build_bass (host-transposed X, full W)."""
    X = np.ascontiguousarray(np.asarray(X, dtype=np.float32))
    rm = np.asarray(running_mean, dtype=np.float32)
    rW = np.ascontiguousarray(np.asarray(running_W, dtype=np.float32))
    rows = X.shape[0] // N_CORES
    bias = (-(rm.astype(np.float64) @ rW.astype(np.float64))).astype(
        np.float32
    ).reshape(1, C)
    return [
        {
            "xt": np.ascontiguousarray(X[c * rows : (c + 1) * rows].T),
            "w": rW,
            "bias": bias,
        }
        for c in range(N_CORES)
    ]


def _prep_in_maps_exact(X, running_mean, running_W):
    """Inputs for build_bass_exact (natural-layout X shards, residual W - I)."""
    X = np.ascontiguousarray(np.asarray(X, dtype=np.float32))
    rm = np.asarray(running_mean, dtype=np.float32)
    rW = np.asarray(running_W, dtype=np.float32)
    rows = X.shape[0] // N_CORES
    r = np.ascontiguousarray(rW - np.eye(C, dtype=np.float32))
    bias = (-(rm.astype(np.float64) @ rW.astype(np.float64))).astype(
        np.float32
    ).reshape(1, C)
    return [
        {
            "x": np.ascontiguousarray(X[c * rows : (c + 1) * rows]),
            "r": r,
            "bias": bias,
        }
        for c in range(N_CORES)
    ]


def kernel(X, running_mean, running_W):
    in_maps = _prep_in_maps_exact(X, running_mean, running_W)
    nc = _CACHE.get("nc")
    if nc is None:
        nc = build_bass_exact()
        _CACHE["nc"] = nc
    res = run_bass_kernel_spmd(nc, in_maps, core_ids=list(range(N_CORES)))
    return np.concatenate([r["out"] for r in res.results], axis=0)



# revision 2
# speedup vs baseline: 2.8694x; 2.8694x over previous
"""Trainium2 Bass kernel for nn_DecorrelatedReNorm_17231408791729.

Math: the reference computes
    out = (X_c @ W @ W_inv + X_mean - running_mean) @ running_W
with W = U diag(S^-1/2) U^T and W_inv = U diag(S^1/2) U^T from eigh(cov).
W @ W_inv == I exactly (same eigenbasis), and X_c + X_mean == X, so
    out = (X - running_mean) @ running_W
identically; the eigh chain contributes only fp32 rounding (~1e-6 rel).

Strategy (data-parallel over N across 8 cores), fully transposed so the
contraction dim (C) rides SBUF partitions with no on-chip transposes:
  - host: shard X rows 8 ways, transpose each shard to X^T [C, rows] and
    round to fp16 (halves HBM traffic; ~3e-4 rel rounding, tol is 2e-2);
    fold running_mean into a per-partition bias b = -(rm @ rW).
  - device (per core): out^T = W^T @ X^T + bias.  lhsT = natural-layout
    W chunks (stationary), rhs = X^T tile (moving).  The scalar engine
    evicts PSUM with the per-partition bias fused; output streams back
    as out^T fp16.
  - variant "t8": residual split out^T = X^T + R^T @ X^T + bias with
    R = W - I quantized to fp8e4 and the matmul in DoubleRow perf mode
    (2x PE throughput); X rides an exact fp16 path so the fp8 rounding
    only touches the residual term (zero when W == I).
  - host: transpose shards back and upcast to fp32.
"""

import numpy as np
from contextlib import ExitStack

import concourse.bass as bass
import concourse.tile as tile
from concourse import bacc, mybir
from concourse.bass_utils import run_bass_kernel_spmd

C = 512
N_ROWS = 131072
N_CORES = 8
ROWS_PER_CORE = N_ROWS // N_CORES  # 16384
P = 128
KC = C // P                        # 4 contraction chunks
R_TILE = 1024                      # rows per macro-tile (DMA granularity)
PS_N = 512                         # rows per PSUM group (one bank)


def build_bass_t(nrows: int = ROWS_PER_CORE, reps: int = 1, r_tile: int = R_TILE):
    """out^T = W^T @ X^T + bias, straight fp16 matmul.

    Exact when W == I (fp16(x)*1.0 products are exact in the fp32 PSUM);
    ~1e-3 rel for general W from fp16 operand rounding.
    """
    f32, f16 = mybir.dt.float32, mybir.dt.float16
    nc = bacc.Bacc(
        "TRN2",
        target_bir_lowering=False,
        debug=False,
        enable_asserts=False,
    )
    xt = nc.dram_tensor("xt", [C, nrows], f16, kind="ExternalInput").ap()
    w = nc.dram_tensor("w", [C, C], f16, kind="ExternalInput").ap()
    b = nc.dram_tensor("bias", [P, KC], f32, kind="ExternalInput").ap()
    ot = nc.dram_tensor("out", [C, nrows], f16, kind="ExternalOutput").ap()

    t_count = nrows // r_tile
    ht = r_tile // PS_N
    # [T, p, kc, r]: partition = c within chunk, free = (chunk, row)
    xt_r = xt.rearrange("(kc p) (t r) -> t p kc r", p=P, r=r_tile)
    ot_r = ot.rearrange("(kc p) (t r) -> t p kc r", p=P, r=r_tile)
    # [p, kc, n]: partition = cin within chunk, free = (chunk, cout)
    w_r = w.rearrange("(kc p) n -> p kc n", p=P)

    with tile.TileContext(nc) as tc, ExitStack() as ctx:
        singles = ctx.enter_context(tc.tile_pool(name="singles", bufs=1))
        xpool = ctx.enter_context(tc.tile_pool(name="x", bufs=3))
        opool = ctx.enter_context(tc.tile_pool(name="o", bufs=3))
        pspool = ctx.enter_context(tc.tile_pool(name="ps", bufs=8, space="PSUM"))

        w_tile = singles.tile([P, KC, C], f16)
        nc.sync.dma_start(out=w_tile[:], in_=w_r)
        bias_tile = singles.tile([P, KC], f32)
        nc.sync.dma_start(out=bias_tile[:], in_=b)

        for _ in range(reps):
            for t in range(t_count):
                x_tile = xpool.tile([P, KC, r_tile], f16, tag="x")
                nc.sync.dma_start(out=x_tile[:], in_=xt_r[t])
                o_tile = opool.tile([P, KC, r_tile], f16, tag="o")
                for j in range(KC):
                    for h in range(ht):
                        ps = pspool.tile([P, PS_N], f32, tag="ps")
                        for k in range(KC):
                            nc.tensor.matmul(
                                ps[:],
                                w_tile[:, k, bass.ts(j, P)],
                                x_tile[:, k, bass.ts(h, PS_N)],
                                start=(k == 0),
                                stop=(k == KC - 1),
                            )
                        nc.scalar.add(
                            o_tile[:, j, bass.ts(h, PS_N)],
                            ps[:],
                            bias_tile[:, j : j + 1],
                        )
                nc.sync.dma_start(out=ot_r[t], in_=o_tile[:])

    nc.compile()
    return nc


def build_bass_t8(nrows: int = ROWS_PER_CORE, reps: int = 1, r_tile: int = R_TILE):
    """out^T = X^T + R^T @ X^T + bias with R = W - I in fp8e4 DoubleRow.

    X rides an exact fp16 path (DVE add); the fp8 truncation only touches
    the residual term, which is exactly zero when W == I.  DoubleRow packs
    the contraction 2-per-cell: each matmul takes lhsT [Ki=128, 2, M] and
    rhs [Ki=128, 2, N], contracting over (Ki x 2) = chunk pairs.
    """
    f32, f16, f8 = mybir.dt.float32, mybir.dt.float16, mybir.dt.float8e4
    nc = bacc.Bacc(
        "TRN2",
        target_bir_lowering=False,
        debug=False,
        enable_asserts=False,
    )
    xt = nc.dram_tensor("xt", [C, nrows], f16, kind="ExternalInput").ap()
    r8 = nc.dram_tensor("r8", [C, C], f8, kind="ExternalInput").ap()
    b = nc.dram_tensor("bias", [P, KC], f32, kind="ExternalInput").ap()
    ot = nc.dram_tensor("out", [C, nrows], f16, kind="ExternalOutput").ap()

    t_count = nrows // r_tile
    ht = r_tile // PS_N
    xt_r = xt.rearrange("(kc p) (t r) -> t p kc r", p=P, r=r_tile)
    ot_r = ot.rearrange("(kc p) (t r) -> t p kc r", p=P, r=r_tile)
    r8_r = r8.rearrange("(kc p) n -> p kc n", p=P)

    with tile.TileContext(nc) as tc, ExitStack() as ctx:
        singles = ctx.enter_context(tc.tile_pool(name="singles", bufs=1))
        xpool = ctx.enter_context(tc.tile_pool(name="x", bufs=3))
        x8pool = ctx.enter_context(tc.tile_pool(name="x8", bufs=3))
        mpool = ctx.enter_context(tc.tile_pool(name="m", bufs=8))
        opool = ctx.enter_context(tc.tile_pool(name="o", bufs=3))
        pspool = ctx.enter_context(tc.tile_pool(name="ps", bufs=8, space="PSUM"))

        r8_tile = singles.tile([P, KC, C], f8)
        nc.sync.dma_start(out=r8_tile[:], in_=r8_r)
        bias_tile = singles.tile([P, KC], f32)
        nc.sync.dma_start(out=bias_tile[:], in_=b)

        for _ in range(reps):
            for t in range(t_count):
                x_tile = xpool.tile([P, KC, r_tile], f16, tag="x")
                nc.sync.dma_start(out=x_tile[:], in_=xt_r[t])
                x8_tile = x8pool.tile([P, KC, r_tile], f8, tag="x8")
                nc.vector.tensor_copy(x8_tile[:], x_tile[:])
                o_tile = opool.tile([P, KC, r_tile], f16, tag="o")
                for j in range(KC):
                    for h in range(ht):
                        ps = pspool.tile([P, PS_N], f32, tag="ps")
                        for g in range(KC // 2):
                            nc.tensor.matmul(
                                ps[:],
                                r8_tile[:, 2 * g : 2 * g + 2, bass.ts(j, P)],
                                x8_tile[:, 2 * g : 2 * g + 2, bass.ts(h, PS_N)],
                                start=(g == 0),
                                stop=(g == KC // 2 - 1),
                                perf_mode=mybir.MatmulPerfMode.DoubleRow,
                            )
                        mid = mpool.tile([P, PS_N], f16, tag="m")
                        nc.scalar.add(mid[:], ps[:], bias_tile[:, j : j + 1])
                        nc.vector.tensor_add(
                            o_tile[:, j, bass.ts(h, PS_N)],
                            mid[:],
                            x_tile[:, j, bass.ts(h, PS_N)],
                        )
                nc.sync.dma_start(out=ot_r[t], in_=o_tile[:])

    nc.compile()
    return nc


def _bias_pp(running_mean, running_W):
    bias = (
        -(
            np.asarray(running_mean, np.float64)
            @ np.asarray(running_W, np.float64)
        )
    ).astype(np.float32)
    return np.ascontiguousarray(bias.reshape(KC, P).T)


def _prep_in_maps_t(X, running_mean, running_W):
    X = np.asarray(X, dtype=np.float32)
    rows = X.shape[0] // N_CORES
    w16 = np.ascontiguousarray(np.asarray(running_W, np.float32).astype(np.float16))
    bias = _bias_pp(running_mean, running_W)
    return [
        {
            "xt": X[c * rows : (c + 1) * rows].T.astype(np.float16),
            "w": w16,
            "bias": bias,
        }
        for c in range(N_CORES)
    ]


def _prep_in_maps_t8(X, running_mean, running_W):
    import ml_dtypes

    X = np.asarray(X, dtype=np.float32)
    rows = X.shape[0] // N_CORES
    r = np.asarray(running_W, np.float32) - np.eye(C, dtype=np.float32)
    r8 = np.ascontiguousarray(r.astype(ml_dtypes.float8_e4m3))
    bias = _bias_pp(running_mean, running_W)
    return [
        {
            "xt": X[c * rows : (c + 1) * rows].T.astype(np.float16),
            "r8": r8,
            "bias": bias,
        }
        for c in range(N_CORES)
    ]


# production variant used by kernel(); test.py times all VARIANTS.
BUILD, PREP = build_bass_t, _prep_in_maps_t
VARIANTS = {
    "t_fp16": (build_bass_t, _prep_in_maps_t),
    "t8_fp8dr": (build_bass_t8, _prep_in_maps_t8),
}

_CACHE: dict = {}


def kernel(X, running_mean, running_W):
    in_maps = PREP(X, running_mean, running_W)
    nc = _CACHE.get("nc")
    if nc is None:
        nc = BUILD()
        _CACHE["nc"] = nc
    res = run_bass_kernel_spmd(nc, in_maps, core_ids=list(range(N_CORES)))
    out = np.empty((N_ROWS, C), np.float32)
    rows = ROWS_PER_CORE
    for c, r in enumerate(res.results):
        out[c * rows : (c + 1) * rows] = r["out"].T
    return out
